# revision 10
# baseline (speedup 1.0000x reference)
"""Trainium2 Bass kernel for nn_GAT_38989713113447 (3-layer dense GAT).

Sharding: 8 heads over 8 cores for the two inner GAT layers (head/tensor
parallel, no communication); AllGather of per-head activations h1T for the
head concat; the output attention layer is sharded over node rows
(384 rows/core) with the per-core slice delivered via AllToAll (so the
SPMD program is identical on every core); the final [3072, 64] output is
assembled host-side from the per-core row slices.

Math: exp(leakyrelu(s)) = max(exp(s), exp(alpha*s)) for alpha in (0,1),
so the [N,N] attention kernel needs one ACT Exp pass (per-partition bias
adds f2_j) max'd with a rank-1 bf16 product via a fused DVE
scalar_tensor_tensor, then a mask multiply.  Attention lives transposed
([j, i], j on partitions) so the PE contracts over j for both the
aggregation matmul and the softmax denominator (ones-matmul).  Masked
entries are exact zeros via the mask multiply; softmax max-subtraction is
skipped (attention logits are O(1)).
"""

import sys

sys.path.insert(0, "/opt/trn_rl_repo")

from contextlib import ExitStack

import numpy as np
import ml_dtypes

import concourse.bass as bass  # noqa: F401
import concourse.bacc as bacc
import concourse.tile as tile
from concourse import mybir
from concourse.bass_utils import run_bass_kernel_spmd

N = 3072
F = 256
H = 8
D = 128          # H1 == H2
OUT = 64
ALPHA = 0.2
NCORES = 8
NJB = N // 128   # 24 attention j-blocks
HALF = N // 2    # i-dim half per PSUM residency
ISL = N // NCORES  # 384 output rows per core

FP32 = mybir.dt.float32
BF16 = mybir.dt.bfloat16
AF = mybir.ActivationFunctionType
ALU = mybir.AluOpType

N_GPSIMD_MASK = 6   # j-blocks whose mask-multiply runs on GPSIMD


def _chunks(total, step):
    return [(o, min(step, total - o)) for o in range(0, total, step)]


class Builder:
    def __init__(self, nc, tc, ctx):
        self.nc = nc
        self.tc = tc
        p = lambda name, bufs, space=None: ctx.enter_context(
            tc.tile_pool(name=name, bufs=bufs, **({"space": space} if space else {}))
        )
        self.state = p("state", 1)
        self.mask = p("mask", 4)
        self.work = p("work", 3)
        self.att = p("att", 4)
        self.ps_agg = p("ps_agg", 1, "PSUM")
        self.ps_rs = p("ps_rs", 1, "PSUM")
        self.ps_sm = p("ps_sm", 2, "PSUM")
        self.misc = p("misc", 1)

    def ones_tile(self, shape, dtype, name):
        t = self.state.tile(shape, dtype, tag=name, name=name)
        self.nc.vector.memset(t[:, :], 1.0)
        return t

    def bcast_row(self, row_ap, width, tag, exp_scale=None):
        """[1, width] bf16 SBUF row -> [128, width] bf16 tile via K=1 matmul.
        With exp_scale, applies Exp(scale*x) on the PSUM->SBUF copy."""
        nc = self.nc
        out = self.state.tile([128, width], BF16, tag=tag)
        for off, w in _chunks(width, 512):
            ps = self.ps_sm.tile([128, 512], FP32, tag="sm", name="sm")
            nc.tensor.matmul(
                ps[:, :w], self.ones1[:, :], row_ap[:, off : off + w],
                start=True, stop=True,
            )
            if exp_scale is None:
                nc.scalar.activation(out[:, off : off + w], ps[:, :w], AF.Copy)
            else:
                nc.scalar.activation(
                    out[:, off : off + w], ps[:, :w], AF.Exp, scale=exp_scale
                )
        return out

    def merge_state(self, xT, seed_row, theta_row):
        """hT[f, i] = xT[f, i] + theta[f] * seed[i] -> bf16 tiles."""
        nc = self.nc
        hT = []
        for fb in range(F // 128):
            ht = self.state.tile([128, N], BF16, tag=f"hT{fb}")
            for off, w in _chunks(N, 512):
                xs = self.misc.tile([128, 512], FP32, tag="xT_stage")
                nc.sync.dma_start(
                    xs[:, :w], xT[fb * 128 : (fb + 1) * 128, off : off + w]
                )
                ps = self.ps_sm.tile([128, 512], FP32, tag="sm", name="sm")
                nc.tensor.matmul(
                    ps[:, :w],
                    theta_row[:, fb * 128 : (fb + 1) * 128],
                    seed_row[:, off : off + w],
                    start=True, stop=True,
                )
                nc.vector.tensor_add(
                    ht[:, off : off + w], xs[:, :w], ps[:, :w]
                )
            hT.append(ht)
        return hT

    def project(self, hT_tiles, w_sb, dT, aug_ones=False):
        """WhT[d, i] (feature-major) and node-major Wh_nm[jb][j, dT(+1)].

        hT_tiles: list of [128, N] bf16 (partitions = features).
        w_sb: matching list of [128, dT] bf16 weight tiles.
        aug_ones: append a ones column to Wh_nm (softmax denom fold)."""
        nc = self.nc
        nk = len(hT_tiles)
        whT = self.state.tile([dT, N], BF16, tag="whT")
        for off, w in _chunks(N, 512):
            ps = self.ps_sm.tile([dT, 512], FP32, tag="sm", name="sm")
            for k in range(nk):
                nc.tensor.matmul(
                    ps[:, :w], w_sb[k][:, :], hT_tiles[k][:, off : off + w],
                    start=(k == 0), stop=(k == nk - 1),
                )
            nc.vector.tensor_copy(whT[:, off : off + w], ps[:, :w])
        wh_nm = []
        wnm = dT + (1 if aug_ones else 0)
        assert wnm <= 128
        for jb in range(NJB):
            ps = self.ps_sm.tile([128, dT], FP32, tag="sm", name="sm")
            for k in range(nk):
                nc.tensor.matmul(
                    ps[:, :],
                    hT_tiles[k][:, jb * 128 : (jb + 1) * 128],
                    w_sb[k][:, :],
                    start=(k == 0), stop=(k == nk - 1),
                )
            t = self.state.tile([128, wnm], BF16, tag=f"whnm_{jb}")
            nc.vector.tensor_copy(t[:, :dT], ps[:, :])
            if aug_ones:
                nc.vector.memset(t[:, dT : dT + 1], 1.0)
            wh_nm.append(t)
        return whT, wh_nm

    def f2_vectors(self, whT, a2_sb, dT):
        """f2col [128, NJB] fp32 and qcol = exp(alpha*f2col)."""
        nc = self.nc
        f2ps = self.ps_sm.tile([128, NJB], FP32, tag="sm", name="sm")
        for jb in range(NJB):
            nc.tensor.matmul(
                f2ps[:, jb : jb + 1],
                whT[:, jb * 128 : (jb + 1) * 128],
                a2_sb[:, :],
                start=True, stop=True,
            )
        f2col = self.state.tile([128, NJB], FP32, tag="f2col")
        nc.scalar.activation(f2col[:, :], f2ps[:, :], AF.Copy)
        qcol = self.state.tile([128, NJB], FP32, tag="qcol")
        nc.scalar.activation(qcol[:, :], f2ps[:, :], AF.Exp, scale=ALPHA)
        return f2col, qcol

    def f1_vectors(self, whT_i, a1_sb, width):
        """f1bc [128, width] bf16 (broadcast f1) and p_bc = exp(alpha*f1)."""
        nc = self.nc
        f1row = self.state.tile([1, width], BF16, tag="f1row")
        for off, w in _chunks(width, 512):
            ps = self.ps_sm.tile([1, 512], FP32, tag="sm", name="sm")
            nc.tensor.matmul(
                ps[:, :w], a1_sb[:, :], whT_i[:, off : off + w],
                start=True, stop=True,
            )
            nc.vector.tensor_copy(f1row[:, off : off + w], ps[:, :w])
        f1bc = self.bcast_row(f1row, width, "f1bc")
        p_bc = self.bcast_row(f1row, width, "p_bc", exp_scale=ALPHA)
        return f1bc, p_bc

    def attention_agg(self, mask_dram, f2col, qcol, f1bc, p_bc, wh_nm, dT,
                      width, h_out, out_elu, fold_rowsum):
        """Masked softmax + aggregation + normalize (+ELU) into h_out."""
        nc = self.nc
        half_w = min(width, HALF)
        for h0 in range(0, width, half_w):
            hw = min(half_w, width - h0)
            nch = len(_chunks(hw, 512))
            arows = dT + 1 if fold_rowsum else dT
            agg_ps = [self.ps_agg.tile([arows, 512], FP32, tag=f"agg{ci}",
                                       name=f"agg{ci}")
                      for ci in range(nch)]
            rs_ps = None
            if not fold_rowsum:
                rs_ps = [self.ps_rs.tile([1, 512], FP32, tag=f"rs{ci}",
                                         name=f"rs{ci}")
                         for ci in range(nch)]
            for jb in range(NJB):
                mt = self.mask.tile([128, hw], BF16, tag="mask")
                nc.sync.dma_start(
                    mt[:, :],
                    mask_dram[jb * 128 : (jb + 1) * 128, h0 : h0 + hw],
                )
                e1 = self.work.tile([128, hw], BF16, tag="e1")
                nc.scalar.activation(
                    e1[:, :], f1bc[:, h0 : h0 + hw], AF.Exp,
                    bias=f2col[:, jb : jb + 1],
                )
                tt = self.work.tile([128, hw], BF16, tag="tt")
                nc.vector.scalar_tensor_tensor(
                    tt[:, :], p_bc[:, h0 : h0 + hw], qcol[:, jb : jb + 1],
                    e1[:, :], ALU.mult, ALU.max,
                )
                at = self.att.tile([128, hw], BF16, tag="at")
                eng = nc.gpsimd if (jb % 4 == 3 and jb // 4 < N_GPSIMD_MASK) \
                    else nc.vector
                eng.tensor_tensor(at[:, :], tt[:, :], mt[:, :], ALU.mult)
                for ci, (off, w) in enumerate(_chunks(hw, 512)):
                    nc.tensor.matmul(
                        agg_ps[ci][:, :w], wh_nm[jb][:, :],
                        at[:, off : off + w],
                        start=(jb == 0), stop=(jb == NJB - 1),
                    )
                if rs_ps is not None:
                    for ci, (off, w) in enumerate(_chunks(hw, 512)):
                        nc.tensor.matmul(
                            rs_ps[ci][:, :w], self.ones128[:, :],
                            at[:, off : off + w],
                            start=(jb == 0), stop=(jb == NJB - 1),
                        )
            if fold_rowsum and getattr(self, "agg_dbg", None) is not None:
                adt = self.misc.tile([dT + 1, hw], FP32, tag="agg_cp", name="agg_cp")
                for ci, (off, w) in enumerate(_chunks(hw, 512)):
                    nc.vector.tensor_copy(adt[:, off : off + w], agg_ps[ci][:, :w])
                nc.sync.dma_start(self.agg_dbg[:, :], adt[:, :])
            # softmax denominator -> reciprocal -> broadcast -> normalize
            rinv = self.misc.tile([1, hw], FP32, tag="rinv")
            for ci, (off, w) in enumerate(_chunks(hw, 512)):
                src = (agg_ps[ci][dT : dT + 1, :w] if fold_rowsum
                       else rs_ps[ci][:, :w])
                nc.vector.reciprocal_approx_fast(rinv[:, off : off + w], src)
            rb_sb = self.misc.tile([dT, hw], FP32, tag="rb_sb")
            for off, w in _chunks(hw, 512):
                ps = self.ps_sm.tile([dT, 512], FP32, tag="sm", name="sm")
                nc.tensor.matmul(
                    ps[:, :w], self.ones1f[:, :dT], rinv[:, off : off + w],
                    start=True, stop=True,
                )
                nc.scalar.activation(rb_sb[:, off : off + w], ps[:, :w], AF.Copy)
            hpn = self.misc.tile([dT, hw], FP32, tag="hpn")
            for ci, (off, w) in enumerate(_chunks(hw, 512)):
                nc.vector.tensor_tensor(
                    hpn[:, off : off + w], agg_ps[ci][:dT, :w],
                    rb_sb[:, off : off + w], ALU.mult,
                )
            if out_elu:
                # ELU(x) = exp(min(x,0)) - 1 + max(x,0)
                m = self.misc.tile([dT, hw], FP32, tag="elu_m")
                nc.vector.tensor_scalar(m[:, :], hpn[:, :], 0.0, None, ALU.min)
                e = self.misc.tile([dT, hw], FP32, tag="elu_e")
                nc.scalar.activation(e[:, :], m[:, :], AF.Exp)
                r = self.misc.tile([dT, hw], FP32, tag="elu_m")
                nc.vector.tensor_scalar(
                    r[:, :], hpn[:, :], 0.0, -1.0, ALU.max, ALU.add
                )
                nc.vector.tensor_add(h_out[:, h0 : h0 + hw], e[:, :], r[:, :])
            else:
                nc.vector.tensor_copy(h_out[:, h0 : h0 + hw], hpn[:, :])


def build(dbg=False):
    nc = bacc.Bacc("TRN2", target_bir_lowering=False, num_devices=NCORES)

    xT = nc.dram_tensor("xT", [F, N], FP32, kind="ExternalInput")
    seed = nc.dram_tensor("seed", [1, N], FP32, kind="ExternalInput")
    theta = nc.dram_tensor("theta", [1, F], FP32, kind="ExternalInput")
    adjT = nc.dram_tensor("adjT", [N, N], BF16, kind="ExternalInput")
    adjT_osl = nc.dram_tensor("adjT_osl", [N, ISL], BF16, kind="ExternalInput")
    w0 = nc.dram_tensor("w0", [F, D], BF16, kind="ExternalInput")
    a01 = nc.dram_tensor("a01", [D, 1], BF16, kind="ExternalInput")
    a02 = nc.dram_tensor("a02", [D, 1], BF16, kind="ExternalInput")
    w1 = nc.dram_tensor("w1", [D, D], BF16, kind="ExternalInput")
    a11 = nc.dram_tensor("a11", [D, 1], BF16, kind="ExternalInput")
    a12 = nc.dram_tensor("a12", [D, 1], BF16, kind="ExternalInput")
    wo = nc.dram_tensor("wo", [H * D, OUT], BF16, kind="ExternalInput")
    ao1 = nc.dram_tensor("ao1", [OUT, 1], BF16, kind="ExternalInput")
    ao2 = nc.dram_tensor("ao2", [OUT, 1], BF16, kind="ExternalInput")

    outT = nc.dram_tensor("outT", [OUT, ISL], FP32, kind="ExternalOutput")
    if dbg:
        h0_dbg = nc.dram_tensor("h0_dbg", [D, N], BF16, kind="ExternalOutput")
        h1_dbg = nc.dram_tensor("h1_dbg", [D, N], BF16, kind="ExternalOutput")
        hc_dbg = nc.dram_tensor("hc_dbg", [2 * D, N], BF16, kind="ExternalOutput")
        hcsl_dbg = nc.dram_tensor("hcsl_dbg", [NCORES * D, ISL], BF16, kind="ExternalOutput")
        whto_dbg = nc.dram_tensor("whto_dbg", [OUT, N], BF16, kind="ExternalOutput")
        whtsl_dbg = nc.dram_tensor("whtsl_dbg", [OUT, ISL], BF16, kind="ExternalOutput")
        f2o_dbg = nc.dram_tensor("f2o_dbg", [128, NJB], FP32, kind="ExternalOutput")
        oraw_dbg = nc.dram_tensor("oraw_dbg", [OUT, ISL], FP32, kind="ExternalOutput")
        whnm_dbg = nc.dram_tensor("whnm_dbg", [128, OUT + 1], BF16, kind="ExternalOutput")
        agg_dbg = nc.dram_tensor("agg_dbg", [OUT + 1, ISL], FP32, kind="ExternalOutput")

    ag_in = nc.dram_tensor("ag_in", [D, N], BF16)
    ag_out = nc.dram_tensor("ag_out", [NCORES * D, N], BF16,
                            addr_space="Shared")
    a2a_in = nc.dram_tensor("a2a_in", [NCORES * D, ISL], BF16)
    a2a_out = nc.dram_tensor("a2a_out", [NCORES * D, ISL], BF16)

    with tile.TileContext(nc) as tc, ExitStack() as ctx:
        b = Builder(nc, tc, ctx)
        b.agg_dbg = None
        b.ones1 = b.ones_tile([1, 128], BF16, "ones1")
        b.ones1f = b.ones_tile([1, 128], FP32, "ones1f")
        b.ones128 = b.ones_tile([128, 1], BF16, "ones128")

        def load_w(ap, shape, tag, dt=BF16):
            s = b.state.tile(shape, dt, tag=tag, name=tag)
            nc.sync.dma_start(s[:, :], ap)
            return s

        theta_sb = load_w(theta[:, :], [1, F], "theta", FP32)
        seed_sb = load_w(seed[:, :], [1, N], "seed", FP32)
        w0_sb = [load_w(w0[k * 128 : (k + 1) * 128, :], [128, D], f"w0_{k}")
                 for k in range(F // 128)]
        a01_sb = load_w(a01[:, :], [D, 1], "a01")
        a02_sb = load_w(a02[:, :], [D, 1], "a02")
        w1_sb = [load_w(w1[:, :], [D, D], "w1")]
        a11_sb = load_w(a11[:, :], [D, 1], "a11")
        a12_sb = load_w(a12[:, :], [D, 1], "a12")
        wo_sb = [load_w(wo[k * 128 : (k + 1) * 128, :], [128, OUT], f"wo_{k}")
                 for k in range(H * D // 128)]
        ao1_sb = load_w(ao1[:, :], [OUT, 1], "ao1")
        ao2_sb = load_w(ao2[:, :], [OUT, 1], "ao2")

        # ---- layer 0 (head h = core id via per-core weight inputs) ----
        hT = b.merge_state(xT, seed_sb, theta_sb)
        whT0, whnm0 = b.project(hT, w0_sb, D)
        f2c0, qc0 = b.f2_vectors(whT0, a02_sb, D)
        f1b0, pb0 = b.f1_vectors(whT0, a01_sb, N)
        h0T = b.state.tile([D, N], BF16, tag="h0T")
        b.attention_agg(adjT, f2c0, qc0, f1b0, pb0, whnm0, D, N, h0T,
                        out_elu=True, fold_rowsum=False)

        # ---- layer 1 ----
        whT1, whnm1 = b.project([h0T], w1_sb, D)
        f2c1, qc1 = b.f2_vectors(whT1, a12_sb, D)
        f1b1, pb1 = b.f1_vectors(whT1, a11_sb, N)
        h1T = b.state.tile([D, N], BF16, tag="h1T")
        b.attention_agg(adjT, f2c1, qc1, f1b1, pb1, whnm1, D, N, h1T,
                        out_elu=True, fold_rowsum=False)

        if dbg:
            nc.sync.dma_start(h0_dbg[:, :], h0T[:, :])
            nc.sync.dma_start(h1_dbg[:, :], h1T[:, :])

        # ---- collectives: full concat (AG) + per-core row slice (A2A) ----
        nc.sync.dma_start(ag_in[:, :], h1T[:, :])
        for j in range(NCORES):
            nc.sync.dma_start(
                a2a_in[j * D : (j + 1) * D, :],
                h1T[:, j * ISL : (j + 1) * ISL],
            )
        nc.gpsimd.collective_compute(
            "AllGather", ALU.bypass,
            replica_groups=[list(range(NCORES))],
            ins=[ag_in.ap().opt()], outs=[ag_out.ap().opt()],
        )
        nc.gpsimd.collective_compute(
            "AllToAll", ALU.bypass,
            replica_groups=[list(range(NCORES))],
            ins=[a2a_in.ap().opt()], outs=[a2a_out.ap().opt()],
        )
        hcT = []
        hc_tags = ["hT0", "hT1", "h0T", "h1T", "hcT4", "hcT5", "hcT6", "hcT7"]
        for k in range(NCORES):
            t = b.state.tile([D, N], BF16, tag=hc_tags[k])
            nc.sync.dma_start(t[:, :], ag_out[k * D : (k + 1) * D, :])
            hcT.append(t)
        hcT_sl = []
        for k in range(NCORES):
            t = b.state.tile([D, ISL], BF16, tag=f"hcSL{k}")
            nc.sync.dma_start(t[:, :], a2a_out[k * D : (k + 1) * D, :])
            hcT_sl.append(t)

        if dbg:
            for k in range(2):
                nc.sync.dma_start(hc_dbg[k * D : (k + 1) * D, :], hcT[k][:, :])
            for k in range(NCORES):
                nc.sync.dma_start(hcsl_dbg[k * D : (k + 1) * D, :], hcT_sl[k][:, :])

        # ---- output attention layer on this core's 384-row slice ----
        whTo, whnmo = b.project(hcT, wo_sb, OUT)
        f2co, qco = b.f2_vectors(whTo, ao2_sb, OUT)
        # f1 side from the A2A slice: WhoT_sl[o, i] for i in core slice
        whTsl = b.state.tile([OUT, ISL], BF16, tag="whTsl")
        ps = b.ps_sm.tile([OUT, ISL], FP32, tag="sm", name="sm")
        for k in range(NCORES):
            nc.tensor.matmul(
                ps[:, :], wo_sb[k][:, :], hcT_sl[k][:, :],
                start=(k == 0), stop=(k == NCORES - 1),
            )
        nc.vector.tensor_copy(whTsl[:, :], ps[:, :])
        if dbg:
            nc.sync.dma_start(whnm_dbg[:, :OUT], whnmo[0][:, :])
            nc.sync.dma_start(whto_dbg[:, :], whTo[:, :])
            nc.sync.dma_start(whtsl_dbg[:, :], whTsl[:, :])
            nc.sync.dma_start(f2o_dbg[:, :], f2co[:, :])
        f1bo, pbo = b.f1_vectors(whTsl, ao1_sb, ISL)
        o_fin = b.state.tile([OUT, ISL], FP32, tag="o_fin")
        b.agg_dbg = agg_dbg if dbg else None
        b.attention_agg(adjT_osl, f2co, qco, f1bo, pbo, whnmo, OUT, ISL,
                        o_fin, out_elu=False, fold_rowsum=False)
        if dbg:
            nc.sync.dma_start(oraw_dbg[:, :], o_fin[:, :])
        # final ELU
        m = b.misc.tile([OUT, ISL], FP32, tag="fin_m")
        nc.vector.tensor_scalar(m[:, :], o_fin[:, :], 0.0, None, ALU.min)
        e = b.misc.tile([OUT, ISL], FP32, tag="fin_e")
        nc.scalar.activation(e[:, :], m[:, :], AF.Exp)
        r = b.misc.tile([OUT, ISL], FP32, tag="fin_r")
        nc.vector.tensor_scalar(r[:, :], o_fin[:, :], 0.0, -1.0, ALU.max,
                                ALU.add)
        fin = b.misc.tile([OUT, ISL], FP32, tag="fin")
        nc.vector.tensor_add(fin[:, :], e[:, :], r[:, :])
        nc.sync.dma_start(outT[:, :], fin[:, :])
    nc.compile()
    return nc


def make_in_maps(inputs):
    x = np.asarray(inputs["x"], np.float32)
    adj = np.asarray(inputs["adj"], np.float32)
    observation = np.asarray(inputs["observation"])
    theta = np.asarray(inputs["theta"], np.float32)
    W0 = np.asarray(inputs["W0"], np.float32)
    a0 = np.asarray(inputs["a0"], np.float32)
    W1 = np.asarray(inputs["W1"], np.float32)
    a1 = np.asarray(inputs["a1"], np.float32)
    Wo = np.asarray(inputs["Wo"], np.float32)
    ao = np.asarray(inputs["ao"], np.float32)

    bf = ml_dtypes.bfloat16
    xT = np.ascontiguousarray(x.T)
    seed = (observation[0] == 1).astype(np.float32)[None, :]
    adjT = np.ascontiguousarray((adj > 0).T.astype(bf))
    wo_bf = Wo.astype(bf)
    ao1 = np.ascontiguousarray(ao[:OUT]).astype(bf)
    ao2 = np.ascontiguousarray(ao[OUT:]).astype(bf)

    in_maps = []
    for c in range(NCORES):
        in_maps.append({
            "xT": xT, "seed": seed, "theta": theta, "adjT": adjT,
            "adjT_osl": np.ascontiguousarray(adjT[:, c * ISL : (c + 1) * ISL]),
            "w0": W0[c].astype(bf),
            "a01": np.ascontiguousarray(a0[c][:D]).astype(bf),
            "a02": np.ascontiguousarray(a0[c][D:]).astype(bf),
            "w1": W1[c].astype(bf),
            "a11": np.ascontiguousarray(a1[c][:D]).astype(bf),
            "a12": np.ascontiguousarray(a1[c][D:]).astype(bf),
            "wo": wo_bf, "ao1": ao1, "ao2": ao2,
        })
    return in_maps


def kernel(**inputs):
    in_maps = make_in_maps(inputs)
    nc = build()
    res = run_bass_kernel_spmd(nc, in_maps, core_ids=list(range(NCORES)))
    out = np.concatenate(
        [res.results[c]["outT"].T for c in range(NCORES)], axis=0
    )
    return np.ascontiguousarray(out, np.float32)


if __name__ == "__main__":
    build()
    print("built ok")
